# revision 1
# baseline (speedup 1.0000x reference)
"""Trainium2 Bass kernel for a 2-layer LSTM (MnistModel frames).

Model: xb [2048, 8192] -> frames [T=64, B, 128] -> LSTM(128->512) ->
LSTM(512->512) -> last hidden -> Linear(512->10).

Sharding: data-parallel over batch (2048 -> 256 per core, 8 cores),
weights replicated.  Everything on-chip lives transposed ([feature,
batch]) so the recurrence needs no transposes; weights are transposed
once at startup via the PE, x is transposed at startup (first few
timesteps kept in SBUF, the rest staged through DRAM scratch).

All matmul operands are bf16 (weights, x, h) - the PE's fast-weight-load
path makes the matmul stream ~25% faster than float32r, and PSUM / cell
state / gate preactivations stay f32 so the rel err stays ~5e-3.  Gate
nonlinearities read PSUM directly on ACT (fused bias); the f32
cell-state ops run on the otherwise-idle GPSIMD Pool engine; the
remaining DVE ops are bf16 in/out (2x DVE rate).  Layer 1 runs LAG
timesteps behind layer 0, the two layers' PSUM group pipelines are
emitted interleaved, and layer 0 is emitted `skew` chunks ahead so the
last L0 gate chunk's h-chain latency hides under L1 matmuls.
"""

import os
import sys

import numpy as np

for _p in ("/opt/trn_rl_repo", "/root/.axon_site/_ro/trn_rl_repo"):
    if os.path.isdir(_p) and _p not in sys.path:
        sys.path.insert(0, _p)

import concourse.bass as bass  # noqa: E402
import concourse.mybir as mybir  # noqa: E402
import concourse.tile as tile  # noqa: E402
from concourse import bacc  # noqa: E402
from concourse.bass_utils import run_bass_kernel_spmd  # noqa: E402
from concourse.masks import make_identity  # noqa: E402

F32 = mybir.dt.float32
F32R = mybir.dt.float32r
AF = mybir.ActivationFunctionType

B, L, IN, H, OUT = 2048, 8192, 128, 512, 10
T = L // IN  # 64 timesteps
NCORES = 8
BL = B // NCORES  # 256 batch rows per core
G4 = 4 * H  # 2048 gate rows
NKC = H // 128  # 4 hidden k-chunks
NMC = G4 // 128  # 16 gate m-chunks
NB = BL  # matmul moving free dim

_CACHE = {}


def _build(opts=None):
    _defaults = dict(
        act_bias="hv2",
        act_bf16=True,
        pool_vc=True,
        wk_bufs=2,
        st_bufs=2,
        xt_bufs=3,
        k_outer=True,
        lag=1,
        xt_sbuf=4,
        bf16=True,  # all matmul operands bf16 (PE fast-weight-load path)
        ps_half=False,  # 1-bank PSUM group tiles, bufs=4 (finer release)
        th_defer=False,  # defer tanh(c)/h-mul one chunk (no ACT HOL block)
        pool_v=False,  # with pool_vc: only v on Pool, c-add on DVE
        skew=1,  # emit layer-0 this many chunks ahead of layer-1
        deep_state=False,  # bufs=3 for c / h1 tiles (break period-2 stalls)
        xt_late=False,  # x-projection matmul second in the L0 group
    )
    _defaults.update(opts or {})
    opts = _defaults
    LAG = opts["lag"]
    assert LAG >= 1, "layer-1 must lag layer-0 by at least one step"
    SKIP = opts["xt_sbuf"]
    # matmul operand dtype: HW forbids mixing 32-bit with 16-bit, so weights,
    # x tiles, h tiles and the head all flip together.
    MMDT = mybir.dt.bfloat16 if opts["bf16"] else F32R

    nc = bacc.Bacc()
    xb = nc.declare_dram_parameter("xb", [BL, L], F32, isOutput=False)
    W_ih0 = nc.declare_dram_parameter("W_ih0", [G4, IN], F32, isOutput=False)
    W_hh0 = nc.declare_dram_parameter("W_hh0", [G4, H], F32, isOutput=False)
    b0 = nc.declare_dram_parameter("b0", [G4], F32, isOutput=False)
    W_ih1 = nc.declare_dram_parameter("W_ih1", [G4, H], F32, isOutput=False)
    W_hh1 = nc.declare_dram_parameter("W_hh1", [G4, H], F32, isOutput=False)
    b1 = nc.declare_dram_parameter("b1", [G4], F32, isOutput=False)
    W_out = nc.declare_dram_parameter("W_out", [OUT, H], F32, isOutput=False)
    b_out = nc.declare_dram_parameter("b_out", [OUT], F32, isOutput=False)
    out = nc.declare_dram_parameter("out", [BL, OUT], F32, isOutput=True)

    with tile.TileContext(nc) as tc:
        with (
            tc.tile_pool(name="const", bufs=1) as const,
            tc.tile_pool(name="xt_dram", bufs=1, space="DRAM") as xdp,
            tc.tile_pool(name="wstg", bufs=2) as wstg,
        ):
            ident = const.tile([128, 128], F32, tag="ident")
            make_identity(nc, ident)

            b0t = const.tile([128, NMC], F32, tag="b0t")
            nc.sync.dma_start(out=b0t, in_=b0[:].rearrange("(m p) -> p m", p=128))
            b1t = const.tile([128, NMC], F32, tag="b1t")
            nc.sync.dma_start(out=b1t, in_=b1[:].rearrange("(m p) -> p m", p=128))
            bot = const.tile([OUT, 1], F32, tag="bot")
            nc.sync.dma_start(out=bot, in_=b_out[:].rearrange("(p o) -> p o", o=1))

            wps_box = {}

            def load_wT(wd, kdim, name, ptile=None):
                """Stage W [4H, kdim] with one big DMA (partition = row%128),
                PE-transpose 128x128 blocks, gather as per-k-chunk [128, 4H]
                float32r tiles (matmul lhsT layout)."""
                nkc = kdim // 128
                wdt = MMDT
                wts = [
                    const.tile(
                        [128, G4], wdt, tag=f"{name}_{kc}", name=f"{name}_{kc}"
                    )
                    for kc in range(nkc)
                ]
                HM = NMC // 4  # stage 4 m-chunks (quarter gate rows) at a time
                wv = wd[:, :].rearrange("(g p) k -> p g k", p=128)
                for half in range(4):
                    st = wstg.tile([128, HM, kdim], F32, tag="st", name=f"st_{name}{half}")
                    # staged[p, g, k] = W[(half*HM + g)*128 + p, k]
                    nc.scalar.dma_start(
                        out=st, in_=wv[:, half * HM : (half + 1) * HM, :]
                    )
                    for mg in range(HM // 4):
                        for kc in range(nkc):
                            pt = (
                                wps_box["p"].tile(
                                    [128, 512], F32, tag="pt", name="wpt"
                                )
                                if ptile is None
                                else ptile()
                            )
                            for j in range(4):
                                mc = mg * 4 + j
                                nc.tensor.transpose(
                                    pt[:, j * 128 : (j + 1) * 128],
                                    st[:, mc, kc * 128 : kc * 128 + 128],
                                    ident,
                                )
                            nc.vector.tensor_copy(
                                wts[kc][
                                    :,
                                    half * HM * 128 + mg * 512 : half * HM * 128
                                    + (mg + 1) * 512,
                                ],
                                pt,
                            )
                return wts

            # x transpose machinery: xb halves staged in column chunks of
            # CHUNK_T timesteps; frames[t].T kept in SBUF for t < SKIP, else
            # round-tripped through DRAM scratch.
            CHUNK_T = 8
            xts_d = [
                None
                if t < SKIP
                else xdp.tile([128, NB], MMDT, tag=f"xt{t}", name=f"xtd{t}")
                for t in range(T)
            ]
            xts_sb = [
                const.tile([128, NB], MMDT, tag=f"xts{t}", name=f"xts{t}")
                if t < SKIP
                else None
                for t in range(T)
            ]

            def x_phase(t_lo, t_hi, xpsum, xstg, xsb):
                for tch in range(t_lo, t_hi, CHUNK_T):
                    tend = min(tch + CHUNK_T, t_hi)
                    lo, hi = tch * IN, tend * IN
                    xs0 = xstg.tile([128, CHUNK_T * IN], F32, tag="xs0", name="xs0")
                    xs1 = xstg.tile([128, CHUNK_T * IN], F32, tag="xs1", name="xs1")
                    nc.sync.dma_start(out=xs0, in_=xb[0:128, lo:hi])
                    nc.sync.dma_start(out=xs1, in_=xb[128:256, lo:hi])
                    for t in range(tch, tend):
                        off = (t - tch) * IN
                        pt = xpsum.tile([128, NB], F32, tag="xpt", name="xpt")
                        nc.tensor.transpose(
                            pt[:, 0:128], xs0[:, off : off + IN], ident
                        )
                        nc.tensor.transpose(
                            pt[:, 128:256], xs1[:, off : off + IN], ident
                        )
                        if t < SKIP:
                            nc.vector.tensor_copy(xts_sb[t], pt)
                        else:
                            sb = xsb.tile([128, NB], MMDT, tag="sb", name="xsb")
                            nc.vector.tensor_copy(sb, pt)
                            nc.sync.dma_start(out=xts_d[t][:, :], in_=sb)

            # ---- startup: layer-0 weights + all x transposes ----
            with (
                tc.tile_pool(name="wpsum", bufs=4, space="PSUM") as wps,
                tc.tile_pool(name="xpsA", bufs=2, space="PSUM") as xpsA,
                tc.tile_pool(name="xstg", bufs=2) as xstg,
                tc.tile_pool(name="xsb", bufs=4) as xsb,
            ):
                wps_box["p"] = wps
                WT_ih0 = load_wT(W_ih0, IN, "wih0")[0]
                WT_hh0 = load_wT(W_hh0, H, "whh0")
                x_phase(0, T, xpsA, xstg, xsb)

            # ---- recurrence ----
            with (
                tc.tile_pool(name="ps0", bufs=2, space="PSUM") as ps0,
                tc.tile_pool(name="ps1", bufs=2, space="PSUM") as ps1,
                tc.tile_pool(name="xtp", bufs=opts["xt_bufs"]) as xtp,
                tc.tile_pool(name="state", bufs=opts["st_bufs"]) as stp,
                tc.tile_pool(name="work", bufs=opts["wk_bufs"]) as wkp,
            ):
                zero = wkp.tile([128, NB], F32, tag="zero")
                nc.vector.memset(zero, 0.0)
                wbox = {}  # one-time loaded weights, shared across reps
                def one_rep(rep):
                    h0, c0, h1, c1 = [], [], [], []
                    for p in range(NKC):
                        for (lst, tg, dt) in (
                            (h0, f"h0_{p}", MMDT),
                            (c0, f"c0_{p}", F32),
                            (h1, f"h1_{p}", MMDT),
                            (c1, f"c1_{p}", F32),
                        ):
                            if tg.startswith("h0"):
                                nb = LAG + 2
                            elif opts["deep_state"] and (
                                tg.startswith("h1") or tg.startswith("c")
                            ):
                                nb = 3
                            else:
                                nb = 2
                            tl = stp.tile([128, NB], dt, tag=tg, name=tg, bufs=nb)
                            if dt != F32:
                                nc.vector.tensor_copy(tl, zero)
                            else:
                                nc.vector.memset(tl, 0.0)
                            lst.append(tl)

                    def lstm_step_gen(lname, pspool, pairs, c_prev, bt, res):
                        """One LSTM layer timestep, transposed layout; yields
                        after each of the NKC gate groups so two layers can be
                        emitted interleaved.

                        PSUM regions hold (i|f|o|g) for one 128-slice of the
                        hidden dim; all `pairs` (wT, rhs) accumulate into them,
                        k-outer so late-arriving rhs chunks are needed late.

                        th_defer: chunk p's tanh(c)/h-mul are emitted at the
                        start of chunk p+1's section so the in-order ACT queue
                        is not head-of-line blocked waiting for c_new."""
                        h_new, c_new = [], []
                        n = len(pairs)
                        ab = opts["act_bias"]
                        ADT = mybir.dt.bfloat16 if opts["act_bf16"] else F32
                        pend = [None]

                        def flush():
                            if pend[0] is not None:
                                pend[0]()
                                pend[0] = None

                        for p in range(NKC):
                            if opts["ps_half"]:
                                psA = pspool.tile(
                                    [128, 2 * NB], F32, tag="gA", name=f"psA{lname}"
                                )
                                psB = pspool.tile(
                                    [128, 2 * NB], F32, tag="gB", name=f"psB{lname}"
                                )
                                regions = [
                                    (psA, 0, 0), (psA, 1, 1),
                                    (psB, 0, 3), (psB, 1, 2),
                                ]  # (tile, slot, gate): i, f, o, g
                                # one bank per tile: a bank's two regions must
                                # not interleave, so emit region-major
                                for (tl, slot, gate) in regions:
                                    mc = gate * NKC + p
                                    for idx, (wt, rhs) in enumerate(pairs):
                                        nc.tensor.matmul(
                                            tl[:, slot * NB : (slot + 1) * NB],
                                            wt[:, mc * 128 : (mc + 1) * 128],
                                            rhs,
                                            start=(idx == 0),
                                            stop=(idx == n - 1),
                                            skip_group_check=True,
                                        )
                                ps_i = psA[:, 0:NB]
                                ps_f = psA[:, NB : 2 * NB]
                                ps_o = psB[:, 0:NB]
                                ps_g = psB[:, NB : 2 * NB]
                            else:
                                ps = pspool.tile(
                                    [128, 4 * NB], F32, tag="g", name=f"ps{lname}"
                                )
                                if opts["k_outer"]:
                                    # k-outer across the two PSUM banks of the
                                    # group tile: regions (i, o) accumulate
                                    # k-interleaved, then (f, g).
                                    for sub in range(2):
                                        for idx, (wt, rhs) in enumerate(pairs):
                                            for pos in (sub, sub + 2):
                                                gate = (0, 1, 3, 2)[pos]
                                                mc = gate * NKC + p
                                                nc.tensor.matmul(
                                                    ps[:, pos * NB : (pos + 1) * NB],
                                                    wt[:, mc * 128 : (mc + 1) * 128],
                                                    rhs,
                                                    start=(idx == 0),
                                                    stop=(idx == n - 1),
                                                    skip_group_check=True,
                                                )
                                else:
                                    for pos, gate in enumerate((0, 1, 3, 2)):
                                        mc = gate * NKC + p
                                        for idx, (wt, rhs) in enumerate(pairs):
                                            nc.tensor.matmul(
                                                ps[:, pos * NB : (pos + 1) * NB],
                                                wt[:, mc * 128 : (mc + 1) * 128],
                                                rhs,
                                                start=(idx == 0),
                                                stop=(idx == n - 1),
                                                skip_group_check=True,
                                            )
                                ps_i = ps[:, 0:NB]
                                ps_f = ps[:, NB : 2 * NB]
                                ps_o = ps[:, 2 * NB : 3 * NB]
                                ps_g = ps[:, 3 * NB : 4 * NB]

                            if opts.get("mm_only"):
                                dmy = wkp.tile([128, NB], F32, tag=f"dmy{lname}")
                                nc.vector.tensor_copy(dmy, ps_i)
                                h_new.append(pairs[-1][1])
                                c_new.append(c_prev[p])
                                yield
                                continue

                            flush()  # previous chunk's deferred th/hn first

                            mci, mcf, mcg, mco = (
                                g * NKC + p for g in (0, 1, 2, 3)
                            )
                            if ab == "hv2":
                                # bias for i,f on DVE, one wide sigmoid on ACT
                                zb = wkp.tile([128, 2 * NB], F32, tag=f"zb{lname}")
                                nc.vector.tensor_scalar_add(
                                    zb[:, 0:NB], ps_i, bt[:, mci : mci + 1]
                                )
                                nc.vector.tensor_scalar_add(
                                    zb[:, NB : 2 * NB], ps_f, bt[:, mcf : mcf + 1]
                                )
                                sgif = wkp.tile(
                                    [128, 2 * NB], ADT, tag=f"sgif{lname}"
                                )
                                nc.scalar.activation(sgif, zb, AF.Sigmoid)
                                sgi = sgif[:, 0:NB]
                                sgf = sgif[:, NB : 2 * NB]
                                tg = wkp.tile([128, NB], ADT, tag=f"tg{lname}")
                                nc.scalar.activation(
                                    tg, ps_g, AF.Tanh, bias=bt[:, mcg : mcg + 1]
                                )
                                sgo = wkp.tile([128, NB], ADT, tag=f"sgo{lname}")
                                nc.scalar.activation(
                                    sgo, ps_o, AF.Sigmoid, bias=bt[:, mco : mco + 1]
                                )
                            else:  # "v2"
                                sgi = wkp.tile([128, NB], ADT, tag=f"sgi{lname}")
                                nc.scalar.activation(
                                    sgi, ps_i, AF.Sigmoid, bias=bt[:, mci : mci + 1]
                                )
                                tg = wkp.tile([128, NB], ADT, tag=f"tg{lname}")
                                nc.scalar.activation(
                                    tg, ps_g, AF.Tanh, bias=bt[:, mcg : mcg + 1]
                                )
                                sgf = wkp.tile([128, NB], F32, tag=f"sgf{lname}")
                                nc.scalar.activation(
                                    sgf, ps_f, AF.Sigmoid, bias=bt[:, mcf : mcf + 1]
                                )
                                sgo = wkp.tile([128, NB], ADT, tag=f"sgo{lname}")
                                nc.scalar.activation(
                                    sgo, ps_o, AF.Sigmoid, bias=bt[:, mco : mco + 1]
                                )
                            u = wkp.tile([128, NB], ADT, tag=f"u{lname}")
                            nc.vector.tensor_mul(u, sgi, tg)
                            veng = nc.gpsimd if opts["pool_vc"] else nc.vector
                            cveng = (
                                nc.gpsimd
                                if (opts["pool_vc"] and not opts.get("pool_v"))
                                else nc.vector
                            )
                            v = wkp.tile([128, NB], F32, tag=f"v{lname}")
                            veng.tensor_mul(v, sgf, c_prev[p])
                            cn = stp.tile(
                                [128, NB], F32, tag=f"c{lname}_{p}",
                                bufs=(3 if opts["deep_state"] else 2),
                            )
                            cveng.tensor_add(cn, u, v)
                            hn = stp.tile(
                                [128, NB],
                                MMDT,
                                tag=f"h{lname}_{p}",
                                bufs=(LAG + 2)
                                if lname == "0"
                                else (3 if opts["deep_state"] else 2),
                            )

                            def fin(cn=cn, hn=hn, sgo=sgo, pp=p):
                                th = wkp.tile([128, NB], ADT, tag=f"th{lname}")
                                nc.scalar.activation(th, cn, AF.Tanh)
                                nc.vector.tensor_mul(hn, sgo, th)

                            if opts["th_defer"] and p < NKC - 1:
                                pend[0] = fin
                            else:
                                fin()
                            h_new.append(hn)
                            c_new.append(cn)
                            yield
                        flush()
                        res[lname] = (h_new, c_new)

                    def drive(gens, skew=0):
                        alive = list(gens)
                        for _ in range(skew):
                            if alive:
                                try:
                                    next(alive[0])
                                except StopIteration:
                                    alive.pop(0)
                        while alive:
                            for g in list(alive):
                                try:
                                    next(g)
                                except StopIteration:
                                    alive.remove(g)

                    hs0 = {}  # t -> h0 chunks (consumed by layer 1 at t)

                    def emit_l0(t):
                        nonlocal h0, c0
                        if xts_sb[t] is not None:
                            xt = xts_sb[t]
                        else:
                            xt = xtp.tile([128, NB], MMDT, tag="xt", name="xt")
                            nc.sync.dma_start(out=xt, in_=xts_d[t][:, :])
                        if opts["xt_late"]:
                            # xt second: its just-in-time DMA no longer gates
                            # the group's start=True matmul
                            pairs = [
                                (WT_hh0[0], h0[0]), (WT_ih0, xt),
                            ] + [(WT_hh0[kc], h0[kc]) for kc in range(1, NKC)]
                        else:
                            pairs = [(WT_ih0, xt)] + [
                                (WT_hh0[kc], h0[kc]) for kc in range(NKC)
                            ]
                        res = {}
                        yield from lstm_step_gen("0", ps0, pairs, c0, b0t, res)
                        h0, c0 = res["0"]
                        hs0[t] = h0

                    def emit_l1(t):
                        nonlocal h1, c1
                        h0t = hs0.pop(t)
                        pairs = [(WT_hh1[kc], h1[kc]) for kc in range(NKC)] + [
                            (WT_ih1[kc], h0t[kc]) for kc in range(NKC)
                        ]
                        res = {}
                        yield from lstm_step_gen("1", ps1, pairs, c1, b1t, res)
                        h1, c1 = res["1"]

                    # layer 0 runs LAG steps ahead (min 1 so layer-1 weights can
                    # stream in while the first L0 step runs); with LAG=0 both
                    # layers of a timestep are emitted interleaved.
                    head_steps = max(LAG, 1)
                    for t in range(head_steps):
                        drive([emit_l0(t)])

                    def ps1_half():
                        if opts["ps_half"]:
                            return ps1.tile(
                                [128, 2 * NB], F32, tag="gA", name="ps1w"
                            )[:, 0:512]
                        return ps1.tile([128, 4 * NB], F32, tag="g", name="ps1w")[
                            :, 0:512
                        ]

                    if rep == 0:
                        wbox["ih1"] = load_wT(W_ih1, H, "wih1", ptile=ps1_half)
                        wbox["hh1"] = load_wT(W_hh1, H, "whh1", ptile=ps1_half)
                    WT_ih1 = wbox["ih1"]
                    WT_hh1 = wbox["hh1"]

                    if LAG == 0:
                        drive([emit_l1(0)])
                    for t in range(head_steps, T):
                        drive(
                            [emit_l0(t), emit_l1(t - LAG)],
                            skew=opts["skew"],
                        )
                    for t in range(T - LAG, T):
                        drive([emit_l1(t)])

                    # head: out.T [10, 256] = W_out @ h1T + b_out
                    if rep == 0:
                        WT_out = const.tile([128, NKC * OUT], MMDT, tag="wout")
                        stw = wstg.tile([OUT, H], F32, tag="st", name="st_wo")
                        nc.scalar.dma_start(out=stw, in_=W_out[:, :])
                        for kc in range(NKC):
                            pt = (
                                ps0.tile([128, 2 * NB], F32, tag="gA", name="ps0w")
                                if opts["ps_half"]
                                else ps0.tile([128, 4 * NB], F32, tag="g", name="ps0w")
                            )[:, 0:OUT]
                            nc.tensor.transpose(
                                pt, stw[:, kc * 128 : (kc + 1) * 128], ident[:OUT, :OUT]
                            )
                            nc.vector.tensor_copy(WT_out[:, kc * OUT : (kc + 1) * OUT], pt)
                        wbox["out"] = WT_out
                    WT_out = wbox["out"]
                    psf = (
                        ps0.tile([128, 2 * NB], F32, tag="gA", name="psf")
                        if opts["ps_half"]
                        else ps0.tile([128, 4 * NB], F32, tag="g", name="psf")
                    )
                    for kc in range(NKC):
                        nc.tensor.matmul(
                            psf[:OUT, 0:NB],
                            WT_out[:, kc * OUT : (kc + 1) * OUT],
                            h1[kc],
                            start=(kc == 0),
                            stop=(kc == NKC - 1),
                        )
                    fo = wkp.tile([128, NB], F32, tag="fo")
                    nc.vector.tensor_scalar_add(
                        fo[:OUT, :], psf[:OUT, 0:NB], bot[:, 0:1]
                    )
                    nc.gpsimd.dma_start(
                        out=out[:, :].rearrange("b o -> o b"), in_=fo[:OUT, :]
                    )

                for rep in range(opts.get("reps", 1)):
                    one_rep(rep)

    nc.compile()
    return nc


def kernel(**inputs):
    if "nc" not in _CACHE:
        _CACHE["nc"] = _build()
    nc = _CACHE["nc"]

    xb = np.asarray(inputs["xb"], dtype=np.float32)
    shared = {
        k: np.ascontiguousarray(np.asarray(inputs[k], dtype=np.float32))
        for k in (
            "W_ih0",
            "W_hh0",
            "b0",
            "W_ih1",
            "W_hh1",
            "b1",
            "W_out",
            "b_out",
        )
    }
    in_maps = []
    for i in range(NCORES):
        m = dict(shared)
        m["xb"] = np.ascontiguousarray(xb[i * BL : (i + 1) * BL])
        in_maps.append(m)

    trace = False
    try:
        trace = bool(int(os.environ.get("KERNEL_TRACE", "0")))
    except ValueError:
        pass
    try:
        res = run_bass_kernel_spmd(nc, in_maps, list(range(NCORES)), trace=trace)
    except ModuleNotFoundError:
        # no NTFF profiling hook in this container; fall back untraced
        res = run_bass_kernel_spmd(nc, in_maps, list(range(NCORES)))
    if trace:
        _CACHE["exec_time_ns"] = res.exec_time_ns
    return np.concatenate(
        [res.results[i]["out"] for i in range(NCORES)], axis=0
    )



# revision 15
# speedup vs baseline: 1.2405x; 1.2405x over previous
"""Trainium2 Bass kernel for a 2-layer LSTM (MnistModel frames).

Model: xb [2048, 8192] -> frames [T=64, B, 128] -> LSTM(128->512) ->
LSTM(512->512) -> last hidden -> Linear(512->10).

Sharding: data-parallel over batch (2048 -> 256 per core, 8 cores),
weights replicated.  Everything on-chip lives transposed ([feature,
batch]) so the recurrence needs no transposes; weights are transposed
once at startup via the PE, x is transposed at startup (first few
timesteps kept in SBUF, the rest staged through DRAM scratch).

The W_hh recurrence matmuls run in fp8e4 DoubleRow mode (2 k-chunks per
PE instruction, 2x the bf16 stream rate); h state is kept as fp8
[128, 2, NB] k-pair tiles, with a bf16 copy of h0 for layer-1's input
matmul (and of h1 at the last step for the head).  x / W_ih / head
matmuls stay bf16: quantizing those pushes rel err past the 2e-2 gate
(measured 1.5e-2 for this split vs 4e-2 all-fp8).  PSUM / cell state /
gate preactivations stay f32.  Gate
nonlinearities read PSUM directly on ACT (fused bias); the f32
cell-state ops run on the otherwise-idle GPSIMD Pool engine; the
remaining DVE ops are bf16 in/out (2x DVE rate).  Layer 1 runs LAG
timesteps behind layer 0, the two layers' PSUM group pipelines are
emitted interleaved, and layer 0 is emitted `skew` chunks ahead so the
last L0 gate chunk's h-chain latency hides under L1 matmuls.
"""

import os
import sys

import numpy as np

for _p in ("/opt/trn_rl_repo", "/root/.axon_site/_ro/trn_rl_repo"):
    if os.path.isdir(_p) and _p not in sys.path:
        sys.path.insert(0, _p)

import concourse.bass as bass  # noqa: E402
import concourse.mybir as mybir  # noqa: E402
import concourse.tile as tile  # noqa: E402
from concourse import bacc  # noqa: E402
from concourse.bass_utils import run_bass_kernel_spmd  # noqa: E402
from concourse.masks import make_identity  # noqa: E402

F32 = mybir.dt.float32
F32R = mybir.dt.float32r
AF = mybir.ActivationFunctionType

B, L, IN, H, OUT = 2048, 8192, 128, 512, 10
T = L // IN  # 64 timesteps
NCORES = 8
BL = B // NCORES  # 256 batch rows per core
G4 = 4 * H  # 2048 gate rows
NKC = H // 128  # 4 hidden k-chunks
NMC = G4 // 128  # 16 gate m-chunks
NB = BL  # matmul moving free dim

_CACHE = {}


def _build(opts=None):
    _defaults = dict(
        act_bias="hv2",
        act_bf16=True,
        pool_vc=True,
        wk_bufs=2,
        st_bufs=2,
        xt_bufs=3,
        k_outer=True,
        lag=1,
        xt_sbuf=4,
        bf16=True,  # all matmul operands bf16 (PE fast-weight-load path)
        ps_half=False,  # 1-bank PSUM group tiles, bufs=4 (finer release)
        th_defer=False,  # defer tanh(c)/h-mul one chunk (no ACT HOL block)
        pool_v=False,  # with pool_vc: only v on Pool, c-add on DVE
        skew=1,  # emit layer-0 this many chunks ahead of layer-1
        deep_state=False,  # bufs=3 for c / h1 tiles (break period-2 stalls)
        xt_late=False,  # x-projection matmul second in the L0 group
        dr=True,  # W_hh matmuls in fp8e4 DoubleRow (2 k-chunks/instr);
        # h kept as fp8 [128, 2, NB] pair tiles.  x / W_ih / head stay
        # bf16 (numerics: fp8 there pushes rel err past the 2e-2 gate).
        l1_ih_first=True,  # L1 group order: ih (h0, LAG-old) before hh (fresh h1)
    )
    _defaults.update(opts or {})
    opts = _defaults
    LAG = opts["lag"]
    assert LAG >= 1, "layer-1 must lag layer-0 by at least one step"
    SKIP = opts["xt_sbuf"]
    # matmul operand dtype: HW forbids mixing 32-bit with 16-bit, so weights,
    # x tiles, h tiles and the head all flip together.
    MMDT = mybir.dt.bfloat16 if opts["bf16"] else F32R
    DR = opts["dr"]
    F8 = mybir.dt.float8e4
    NPR = NKC // 2  # hidden k-chunk pairs per layer (DoubleRow granularity)

    nc = bacc.Bacc()
    xb = nc.declare_dram_parameter("xb", [BL, L], F32, isOutput=False)
    W_ih0 = nc.declare_dram_parameter("W_ih0", [G4, IN], F32, isOutput=False)
    W_hh0 = nc.declare_dram_parameter("W_hh0", [G4, H], F32, isOutput=False)
    b0 = nc.declare_dram_parameter("b0", [G4], F32, isOutput=False)
    W_ih1 = nc.declare_dram_parameter("W_ih1", [G4, H], F32, isOutput=False)
    W_hh1 = nc.declare_dram_parameter("W_hh1", [G4, H], F32, isOutput=False)
    b1 = nc.declare_dram_parameter("b1", [G4], F32, isOutput=False)
    W_out = nc.declare_dram_parameter("W_out", [OUT, H], F32, isOutput=False)
    b_out = nc.declare_dram_parameter("b_out", [OUT], F32, isOutput=False)
    out = nc.declare_dram_parameter("out", [BL, OUT], F32, isOutput=True)

    with tile.TileContext(nc) as tc:
        with (
            tc.tile_pool(name="const", bufs=1) as const,
            tc.tile_pool(name="xt_dram", bufs=1, space="DRAM") as xdp,
            tc.tile_pool(name="wstg", bufs=2) as wstg,
        ):
            ident = const.tile([128, 128], F32, tag="ident")
            make_identity(nc, ident)

            b0t = const.tile([128, NMC], F32, tag="b0t")
            nc.sync.dma_start(out=b0t, in_=b0[:].rearrange("(m p) -> p m", p=128))
            b1t = const.tile([128, NMC], F32, tag="b1t")
            nc.sync.dma_start(out=b1t, in_=b1[:].rearrange("(m p) -> p m", p=128))
            bot = const.tile([OUT, 1], F32, tag="bot")
            nc.sync.dma_start(out=bot, in_=b_out[:].rearrange("(p o) -> p o", o=1))

            wps_box = {}

            def load_wT(wd, kdim, name, ptile=None, pair=False):
                """Stage W [4H, kdim] with one big DMA (partition = row%128),
                PE-transpose 128x128 blocks, gather as per-k-chunk [128, 4H]
                tiles (matmul lhsT layout).  pair=True instead gathers fp8e4
                [128, 2, 4H] k-pair tiles (DoubleRow lhsT layout: dim1 =
                k-plane within the pair)."""
                nkc = kdim // 128
                if pair:
                    wts = [
                        const.tile(
                            [128, 2, G4], F8, tag=f"{name}_{j}", name=f"{name}_{j}"
                        )
                        for j in range(nkc // 2)
                    ]
                else:
                    wts = [
                        const.tile(
                            [128, G4], MMDT, tag=f"{name}_{kc}", name=f"{name}_{kc}"
                        )
                        for kc in range(nkc)
                    ]
                HM = NMC // 4  # stage 4 m-chunks (quarter gate rows) at a time
                wv = wd[:, :].rearrange("(g p) k -> p g k", p=128)
                for half in range(4):
                    st = wstg.tile([128, HM, kdim], F32, tag="st", name=f"st_{name}{half}")
                    # staged[p, g, k] = W[(half*HM + g)*128 + p, k]
                    nc.scalar.dma_start(
                        out=st, in_=wv[:, half * HM : (half + 1) * HM, :]
                    )
                    for mg in range(HM // 4):
                        for kc in range(nkc):
                            pt = (
                                wps_box["p"].tile(
                                    [128, 512], F32, tag="pt", name="wpt"
                                )
                                if ptile is None
                                else ptile()
                            )
                            for j in range(4):
                                mc = mg * 4 + j
                                nc.tensor.transpose(
                                    pt[:, j * 128 : (j + 1) * 128],
                                    st[:, mc, kc * 128 : kc * 128 + 128],
                                    ident,
                                )
                            lo = half * HM * 128 + mg * 512
                            if pair:
                                nc.vector.tensor_copy(
                                    wts[kc // 2][:, kc % 2, lo : lo + 512], pt
                                )
                            else:
                                nc.vector.tensor_copy(
                                    wts[kc][:, lo : lo + 512], pt
                                )
                return wts

            # x transpose machinery: xb halves staged in column chunks of
            # CHUNK_T timesteps; frames[t].T kept in SBUF for t < SKIP, else
            # round-tripped through DRAM scratch.
            CHUNK_T = 8
            xts_d = [
                None
                if t < SKIP
                else xdp.tile([128, NB], MMDT, tag=f"xt{t}", name=f"xtd{t}")
                for t in range(T)
            ]
            xts_sb = [
                const.tile([128, NB], MMDT, tag=f"xts{t}", name=f"xts{t}")
                if t < SKIP
                else None
                for t in range(T)
            ]

            def x_phase(t_lo, t_hi, xpsum, xstg, xsb):
                for tch in range(t_lo, t_hi, CHUNK_T):
                    tend = min(tch + CHUNK_T, t_hi)
                    lo, hi = tch * IN, tend * IN
                    xs0 = xstg.tile([128, CHUNK_T * IN], F32, tag="xs0", name="xs0")
                    xs1 = xstg.tile([128, CHUNK_T * IN], F32, tag="xs1", name="xs1")
                    nc.sync.dma_start(out=xs0, in_=xb[0:128, lo:hi])
                    nc.sync.dma_start(out=xs1, in_=xb[128:256, lo:hi])
                    for t in range(tch, tend):
                        off = (t - tch) * IN
                        pt = xpsum.tile([128, NB], F32, tag="xpt", name="xpt")
                        nc.tensor.transpose(
                            pt[:, 0:128], xs0[:, off : off + IN], ident
                        )
                        nc.tensor.transpose(
                            pt[:, 128:256], xs1[:, off : off + IN], ident
                        )
                        if t < SKIP:
                            nc.vector.tensor_copy(xts_sb[t], pt)
                        else:
                            sb = xsb.tile([128, NB], MMDT, tag="sb", name="xsb")
                            nc.vector.tensor_copy(sb, pt)
                            nc.sync.dma_start(out=xts_d[t][:, :], in_=sb)

            # ---- startup: layer-0 weights + all x transposes ----
            with (
                tc.tile_pool(name="wpsum", bufs=4, space="PSUM") as wps,
                tc.tile_pool(name="xpsA", bufs=2, space="PSUM") as xpsA,
                tc.tile_pool(name="xstg", bufs=2) as xstg,
                tc.tile_pool(name="xsb", bufs=4) as xsb,
            ):
                wps_box["p"] = wps
                WT_ih0 = load_wT(W_ih0, IN, "wih0")[0]
                WT_hh0 = load_wT(W_hh0, H, "whh0", pair=DR)
                x_phase(0, T, xpsA, xstg, xsb)

            # ---- recurrence ----
            with (
                tc.tile_pool(name="ps0", bufs=2, space="PSUM") as ps0,
                tc.tile_pool(name="ps1", bufs=2, space="PSUM") as ps1,
                tc.tile_pool(name="xtp", bufs=opts["xt_bufs"]) as xtp,
                tc.tile_pool(name="state", bufs=opts["st_bufs"]) as stp,
                tc.tile_pool(name="work", bufs=opts["wk_bufs"]) as wkp,
            ):
                zero = wkp.tile([128, NB], F32, tag="zero")
                nc.vector.memset(zero, 0.0)
                wbox = {}  # one-time loaded weights, shared across reps
                def one_rep(rep):
                    h0, c0, h1, c1 = [], [], [], []
                    hp0, hp1 = [], []
                    for p in range(NKC):
                        for (lst, tg, dt) in (
                            (h0, f"h0_{p}", MMDT),
                            (c0, f"c0_{p}", F32),
                            (h1, f"h1_{p}", MMDT),
                            (c1, f"c1_{p}", F32),
                        ):
                            if DR and tg.startswith("h"):
                                continue  # DR recurrence reads hp pairs only
                            if tg.startswith("h0"):
                                nb = LAG + 2
                            elif opts["deep_state"] and (
                                tg.startswith("h1") or tg.startswith("c")
                            ):
                                nb = 3
                            else:
                                nb = 2
                            tl = stp.tile([128, NB], dt, tag=tg, name=tg, bufs=nb)
                            if dt != F32:
                                nc.vector.tensor_copy(tl, zero)
                            else:
                                nc.vector.memset(tl, 0.0)
                            lst.append(tl)
                    if DR:
                        for (lst, ln) in ((hp0, "0"), (hp1, "1")):
                            for j in range(NPR):
                                tl = stp.tile(
                                    [128, 2, NB], F8, tag=f"hp{ln}_{j}",
                                    name=f"hp{ln}_{j}", bufs=2,
                                )
                                nc.vector.tensor_copy(tl[:, 0, :], zero)
                                nc.vector.tensor_copy(tl[:, 1, :], zero)
                                lst.append(tl)

                    def lstm_step_gen(lname, pspool, pairs, c_prev, bt, res,
                                      h_bf=True):
                        """One LSTM layer timestep, transposed layout; yields
                        after each of the NKC gate groups so two layers can be
                        emitted interleaved.

                        PSUM regions hold (i|f|o|g) for one 128-slice of the
                        hidden dim; all `pairs` (wT, rhs) accumulate into them,
                        k-outer so late-arriving rhs chunks are needed late.

                        th_defer: chunk p's tanh(c)/h-mul are emitted at the
                        start of chunk p+1's section so the in-order ACT queue
                        is not head-of-line blocked waiting for c_new.

                        `pairs` entries are ("mm", wT, rhs) single-k-chunk
                        bf16 matmuls or ("dr", wpair, hpair) fp8 DoubleRow
                        matmuls covering 2 k-chunks.  DR mode writes h into
                        fp8 [128, 2, NB] pair tiles (hp_new); bf16 h chunks
                        are additionally produced when h_bf (L0 every step:
                        L1's input matmul reads them; L1 only at t=T-1 for
                        the head)."""
                        h_new, c_new = [], []
                        n = len(pairs)
                        ab = opts["act_bias"]
                        ADT = mybir.dt.bfloat16 if opts["act_bf16"] else F32
                        pend = [None]
                        hp_new = None
                        if DR:
                            hp_new = [
                                stp.tile(
                                    [128, 2, NB], F8, tag=f"hp{lname}_{j}",
                                    name=f"hp{lname}_{j}", bufs=2,
                                )
                                for j in range(NPR)
                            ]

                        def flush():
                            if pend[0] is not None:
                                pend[0]()
                                pend[0] = None

                        def emit_mm(region, gate, p, idx, entry):
                            mode, wt, rhs = entry
                            mc = gate * NKC + p
                            if mode == "dr":
                                nc.tensor.matmul(
                                    region,
                                    wt[:, :, mc * 128 : (mc + 1) * 128],
                                    rhs,
                                    start=(idx == 0),
                                    stop=(idx == n - 1),
                                    perf_mode=mybir.MatmulPerfMode.DoubleRow,
                                    skip_group_check=True,
                                )
                            else:
                                nc.tensor.matmul(
                                    region,
                                    wt[:, mc * 128 : (mc + 1) * 128],
                                    rhs,
                                    start=(idx == 0),
                                    stop=(idx == n - 1),
                                    skip_group_check=True,
                                )

                        for p in range(NKC):
                            if opts["ps_half"]:
                                psA = pspool.tile(
                                    [128, 2 * NB], F32, tag="gA", name=f"psA{lname}"
                                )
                                psB = pspool.tile(
                                    [128, 2 * NB], F32, tag="gB", name=f"psB{lname}"
                                )
                                regions = [
                                    (psA, 0, 0), (psA, 1, 1),
                                    (psB, 0, 3), (psB, 1, 2),
                                ]  # (tile, slot, gate): i, f, o, g
                                # one bank per tile: a bank's two regions must
                                # not interleave, so emit region-major
                                for (tl, slot, gate) in regions:
                                    for idx, entry in enumerate(pairs):
                                        emit_mm(
                                            tl[:, slot * NB : (slot + 1) * NB],
                                            gate, p, idx, entry,
                                        )
                                ps_i = psA[:, 0:NB]
                                ps_f = psA[:, NB : 2 * NB]
                                ps_o = psB[:, 0:NB]
                                ps_g = psB[:, NB : 2 * NB]
                            else:
                                ps = pspool.tile(
                                    [128, 4 * NB], F32, tag="g", name=f"ps{lname}"
                                )
                                if opts["k_outer"]:
                                    # k-outer across the two PSUM banks of the
                                    # group tile: regions (i, o) accumulate
                                    # k-interleaved, then (f, g).
                                    for sub in range(2):
                                        for idx, entry in enumerate(pairs):
                                            for pos in (sub, sub + 2):
                                                gate = (0, 1, 3, 2)[pos]
                                                emit_mm(
                                                    ps[:, pos * NB : (pos + 1) * NB],
                                                    gate, p, idx, entry,
                                                )
                                else:
                                    for pos, gate in enumerate((0, 1, 3, 2)):
                                        for idx, entry in enumerate(pairs):
                                            emit_mm(
                                                ps[:, pos * NB : (pos + 1) * NB],
                                                gate, p, idx, entry,
                                            )
                                ps_i = ps[:, 0:NB]
                                ps_f = ps[:, NB : 2 * NB]
                                ps_o = ps[:, 2 * NB : 3 * NB]
                                ps_g = ps[:, 3 * NB : 4 * NB]

                            if opts.get("mm_only"):
                                dmy = wkp.tile([128, NB], F32, tag=f"dmy{lname}")
                                nc.vector.tensor_copy(dmy, ps_i)
                                h_new.append(pairs[-1][2])
                                c_new.append(c_prev[p])
                                yield
                                continue

                            flush()  # previous chunk's deferred th/hn first

                            mci, mcf, mcg, mco = (
                                g * NKC + p for g in (0, 1, 2, 3)
                            )
                            if ab == "hv2":
                                # bias for i,f on DVE, one wide sigmoid on ACT
                                zb = wkp.tile([128, 2 * NB], F32, tag=f"zb{lname}")
                                nc.vector.tensor_scalar_add(
                                    zb[:, 0:NB], ps_i, bt[:, mci : mci + 1]
                                )
                                nc.vector.tensor_scalar_add(
                                    zb[:, NB : 2 * NB], ps_f, bt[:, mcf : mcf + 1]
                                )
                                sgif = wkp.tile(
                                    [128, 2 * NB], ADT, tag=f"sgif{lname}"
                                )
                                nc.scalar.activation(sgif, zb, AF.Sigmoid)
                                sgi = sgif[:, 0:NB]
                                sgf = sgif[:, NB : 2 * NB]
                                tg = wkp.tile([128, NB], ADT, tag=f"tg{lname}")
                                nc.scalar.activation(
                                    tg, ps_g, AF.Tanh, bias=bt[:, mcg : mcg + 1]
                                )
                                sgo = wkp.tile([128, NB], ADT, tag=f"sgo{lname}")
                                nc.scalar.activation(
                                    sgo, ps_o, AF.Sigmoid, bias=bt[:, mco : mco + 1]
                                )
                            else:  # "v2"
                                sgi = wkp.tile([128, NB], ADT, tag=f"sgi{lname}")
                                nc.scalar.activation(
                                    sgi, ps_i, AF.Sigmoid, bias=bt[:, mci : mci + 1]
                                )
                                tg = wkp.tile([128, NB], ADT, tag=f"tg{lname}")
                                nc.scalar.activation(
                                    tg, ps_g, AF.Tanh, bias=bt[:, mcg : mcg + 1]
                                )
                                sgf = wkp.tile([128, NB], F32, tag=f"sgf{lname}")
                                nc.scalar.activation(
                                    sgf, ps_f, AF.Sigmoid, bias=bt[:, mcf : mcf + 1]
                                )
                                sgo = wkp.tile([128, NB], ADT, tag=f"sgo{lname}")
                                nc.scalar.activation(
                                    sgo, ps_o, AF.Sigmoid, bias=bt[:, mco : mco + 1]
                                )
                            u = wkp.tile([128, NB], ADT, tag=f"u{lname}")
                            nc.vector.tensor_mul(u, sgi, tg)
                            veng = nc.gpsimd if opts["pool_vc"] else nc.vector
                            cveng = (
                                nc.gpsimd
                                if (opts["pool_vc"] and not opts.get("pool_v"))
                                else nc.vector
                            )
                            v = wkp.tile([128, NB], F32, tag=f"v{lname}")
                            veng.tensor_mul(v, sgf, c_prev[p])
                            cn = stp.tile(
                                [128, NB], F32, tag=f"c{lname}_{p}",
                                bufs=(3 if opts["deep_state"] else 2),
                            )
                            cveng.tensor_add(cn, u, v)
                            hn = None
                            if not DR or h_bf:
                                hn = stp.tile(
                                    [128, NB],
                                    MMDT,
                                    tag=f"h{lname}_{p}",
                                    bufs=(LAG + 2)
                                    if lname == "0"
                                    else (3 if opts["deep_state"] else 2),
                                )

                            def fin(cn=cn, hn=hn, sgo=sgo, pp=p):
                                th = wkp.tile([128, NB], ADT, tag=f"th{lname}")
                                nc.scalar.activation(th, cn, AF.Tanh)
                                if hn is not None:
                                    nc.vector.tensor_mul(hn, sgo, th)
                                    if DR:
                                        nc.vector.tensor_copy(
                                            hp_new[pp // 2][:, pp % 2, :], hn
                                        )
                                else:
                                    nc.vector.tensor_mul(
                                        hp_new[pp // 2][:, pp % 2, :], sgo, th
                                    )

                            if opts["th_defer"] and p < NKC - 1:
                                pend[0] = fin
                            else:
                                fin()
                            h_new.append(hn)
                            c_new.append(cn)
                            yield
                        flush()
                        res[lname] = (h_new, hp_new, c_new)

                    def drive(gens, skew=0):
                        alive = list(gens)
                        for _ in range(skew):
                            if alive:
                                try:
                                    next(alive[0])
                                except StopIteration:
                                    alive.pop(0)
                        while alive:
                            for g in list(alive):
                                try:
                                    next(g)
                                except StopIteration:
                                    alive.remove(g)

                    hs0 = {}  # t -> h0 chunks (consumed by layer 1 at t)

                    def emit_l0(t):
                        nonlocal h0, c0, hp0
                        if xts_sb[t] is not None:
                            xt = xts_sb[t]
                        else:
                            xt = xtp.tile([128, NB], MMDT, tag="xt", name="xt")
                            nc.sync.dma_start(out=xt, in_=xts_d[t][:, :])
                        if DR:
                            hh = [("dr", WT_hh0[j], hp0[j]) for j in range(NPR)]
                            if opts["xt_late"]:
                                pairs = [hh[0], ("mm", WT_ih0, xt)] + hh[1:]
                            else:
                                pairs = [("mm", WT_ih0, xt)] + hh
                        elif opts["xt_late"]:
                            # xt second: its just-in-time DMA no longer gates
                            # the group's start=True matmul
                            pairs = [
                                ("mm", WT_hh0[0], h0[0]), ("mm", WT_ih0, xt),
                            ] + [("mm", WT_hh0[kc], h0[kc]) for kc in range(1, NKC)]
                        else:
                            pairs = [("mm", WT_ih0, xt)] + [
                                ("mm", WT_hh0[kc], h0[kc]) for kc in range(NKC)
                            ]
                        res = {}
                        yield from lstm_step_gen("0", ps0, pairs, c0, b0t, res)
                        h0, hp0, c0 = res["0"]
                        hs0[t] = h0

                    def emit_l1(t):
                        nonlocal h1, c1, hp1
                        h0t = hs0.pop(t)
                        if DR:
                            ih = [("mm", WT_ih1[kc], h0t[kc]) for kc in range(NKC)]
                            hh = [("dr", WT_hh1[j], hp1[j]) for j in range(NPR)]
                            pairs = (
                                ih + hh if opts["l1_ih_first"] else hh + ih
                            )
                        else:
                            pairs = [
                                ("mm", WT_hh1[kc], h1[kc]) for kc in range(NKC)
                            ] + [("mm", WT_ih1[kc], h0t[kc]) for kc in range(NKC)]
                        res = {}
                        yield from lstm_step_gen(
                            "1", ps1, pairs, c1, b1t, res,
                            h_bf=(not DR) or (t == T - 1),
                        )
                        h1, hp1, c1 = res["1"]

                    # layer 0 runs LAG steps ahead (min 1 so layer-1 weights can
                    # stream in while the first L0 step runs); with LAG=0 both
                    # layers of a timestep are emitted interleaved.
                    head_steps = max(LAG, 1)
                    for t in range(head_steps):
                        drive([emit_l0(t)])

                    def ps1_half():
                        if opts["ps_half"]:
                            return ps1.tile(
                                [128, 2 * NB], F32, tag="gA", name="ps1w"
                            )[:, 0:512]
                        return ps1.tile([128, 4 * NB], F32, tag="g", name="ps1w")[
                            :, 0:512
                        ]

                    if rep == 0:
                        wbox["ih1"] = load_wT(W_ih1, H, "wih1", ptile=ps1_half)
                        wbox["hh1"] = load_wT(
                            W_hh1, H, "whh1", ptile=ps1_half, pair=DR
                        )
                    WT_ih1 = wbox["ih1"]
                    WT_hh1 = wbox["hh1"]

                    if LAG == 0:
                        drive([emit_l1(0)])
                    for t in range(head_steps, T):
                        drive(
                            [emit_l0(t), emit_l1(t - LAG)],
                            skew=opts["skew"],
                        )
                    for t in range(T - LAG, T):
                        drive([emit_l1(t)])

                    # head: out.T [10, 256] = W_out @ h1T + b_out
                    if rep == 0:
                        WT_out = const.tile([128, NKC * OUT], MMDT, tag="wout")
                        stw = wstg.tile([OUT, H], F32, tag="st", name="st_wo")
                        nc.scalar.dma_start(out=stw, in_=W_out[:, :])
                        for kc in range(NKC):
                            pt = (
                                ps0.tile([128, 2 * NB], F32, tag="gA", name="ps0w")
                                if opts["ps_half"]
                                else ps0.tile([128, 4 * NB], F32, tag="g", name="ps0w")
                            )[:, 0:OUT]
                            nc.tensor.transpose(
                                pt, stw[:, kc * 128 : (kc + 1) * 128], ident[:OUT, :OUT]
                            )
                            nc.vector.tensor_copy(WT_out[:, kc * OUT : (kc + 1) * OUT], pt)
                        wbox["out"] = WT_out
                    WT_out = wbox["out"]
                    psf = (
                        ps0.tile([128, 2 * NB], F32, tag="gA", name="psf")
                        if opts["ps_half"]
                        else ps0.tile([128, 4 * NB], F32, tag="g", name="psf")
                    )
                    for kc in range(NKC):
                        nc.tensor.matmul(
                            psf[:OUT, 0:NB],
                            WT_out[:, kc * OUT : (kc + 1) * OUT],
                            h1[kc],
                            start=(kc == 0),
                            stop=(kc == NKC - 1),
                        )
                    fo = wkp.tile([128, NB], F32, tag="fo")
                    nc.vector.tensor_scalar_add(
                        fo[:OUT, :], psf[:OUT, 0:NB], bot[:, 0:1]
                    )
                    nc.gpsimd.dma_start(
                        out=out[:, :].rearrange("b o -> o b"), in_=fo[:OUT, :]
                    )

                for rep in range(opts.get("reps", 1)):
                    one_rep(rep)

    nc.compile()
    return nc


def kernel(**inputs):
    if "nc" not in _CACHE:
        _CACHE["nc"] = _build()
    nc = _CACHE["nc"]

    xb = np.asarray(inputs["xb"], dtype=np.float32)
    shared = {
        k: np.ascontiguousarray(np.asarray(inputs[k], dtype=np.float32))
        for k in (
            "W_ih0",
            "W_hh0",
            "b0",
            "W_ih1",
            "W_hh1",
            "b1",
            "W_out",
            "b_out",
        )
    }
    in_maps = []
    for i in range(NCORES):
        m = dict(shared)
        m["xb"] = np.ascontiguousarray(xb[i * BL : (i + 1) * BL])
        in_maps.append(m)

    trace = False
    try:
        trace = bool(int(os.environ.get("KERNEL_TRACE", "0")))
    except ValueError:
        pass
    try:
        res = run_bass_kernel_spmd(nc, in_maps, list(range(NCORES)), trace=trace)
    except ModuleNotFoundError:
        # no NTFF profiling hook in this container; fall back untraced
        res = run_bass_kernel_spmd(nc, in_maps, list(range(NCORES)))
    if trace:
        _CACHE["exec_time_ns"] = res.exec_time_ns
    return np.concatenate(
        [res.results[i]["out"] for i in range(NCORES)], axis=0
    )

